# revision 28
# baseline (speedup 1.0000x reference)
"""BERT-base (12-layer, B=8, S=512, D=768, H=12, F=3072) forward pass on 8
Trainium2 NeuronCores.

Strategy: data-parallel over batch (1 sequence per core, no collectives).
Per core, activations are kept feature-major xT[D, S] in SBUF so that:
  - all big matmuls run as lhsT[dK,128] @ rhs[dK,512] at full PE rate
  - LayerNorm statistics (sums over the feature dim = partition dim) are
    ones-vector matmuls on the PE
  - softmax runs in scoresT [k, q] layout: the attention mask is a
    per-partition exp() bias, and denominators fall out of a ones-augmented
    V column in the P@V matmul (M=65)
  - per-column broadcasts (mu, rstd, 1/denom) are rank-1 ones-matmuls
v2 changes vs baseline:
  - QKV + attention + Wo emission interleaved so ACT exp overlaps PE work
    (keeps the PE dense -> HAM stays at full clock)
  - denominator folded into PV (ones-augmented V column) - kills 24
    ones-matmuls/layer
  - score matmuls (K=64) packed pairwise into disjoint PE row groups via
    tile_position -> 2x concurrency
  - host-side weight relayout: every weight tile is ONE dma_start with
    1.5-3KB contiguous per-partition lines
  - LN1 gamma/beta folded into W1 (host), b1 folded into b2 (host)
  - exp-tile pool 16-deep; LN scalar chains bridged with warm matmuls
  - deterministic 8-bank PSUM tag plan
"""
import numpy as np

import concourse.bass as bass
import concourse.mybir as mybir
import concourse.tile as tile
from concourse import bass_utils
from concourse.masks import make_identity

AF = mybir.ActivationFunctionType
OP = mybir.AluOpType
F32 = mybir.dt.float32
F32R = mybir.dt.float32r
BF16 = mybir.dt.bfloat16
I32 = mybir.dt.int32

B, S, D, H, F, L, V = 8, 512, 768, 12, 3072, 12, 30522
DK = D // H
SCALE = 1.0 / float(np.sqrt(DK))
NT = D // 128      # 6 feature tiles
NTF = F // 128     # 24 ffn tiles
NST = S // 128     # 4 sequence tiles
NP = H // 2        # 6 head pairs

_NC_CACHE = None


# ---------------------------------------------------------------------------
# wait-slot legalization: walrus codegen allows only ONE sync-wait command on
# TPB instructions; hoist excess waits into standalone EventSemaphores.
def _legalize_waits(nc):
    skip = (mybir.InstEventSemaphore, mybir.InstNoOp)
    n = 0
    for fn in nc.m.functions:
        for blk in fn.blocks:
            out = []
            for inst in blk.instructions:
                si = inst.sync_info
                if si is not None and si.on_wait and not isinstance(inst, skip) \
                        and len(si.on_wait) > 1:
                    waits = list(si.on_wait)
                    for j, w in enumerate(waits[:-1]):
                        ev = mybir.InstEventSemaphore(
                            name=f"{inst.name}-lgw{j}", ins=[], outs=[],
                            sync_info=mybir.SyncInfo(on_wait=[w], on_update=[]),
                        )
                        ev.engine = inst.engine
                        out.append(ev)
                        n += 1
                    inst.sync_info = mybir.SyncInfo(
                        on_wait=[waits[-1]], on_update=list(si.on_update))
                out.append(inst)
            try:
                blk.instructions = out
            except Exception:
                blk.instructions.clear()
                blk.instructions.extend(out)
    return n


def _build_nc():
    nc = bass.Bass("TRN2", target_bir_lowering=False, debug=False,
                   enable_asserts=False, num_devices=8)

    # ---- DRAM I/O ---------------------------------------------------------
    d_ids = nc.dram_tensor("ids", [S, 1], I32, kind="ExternalInput")
    d_tti = nc.dram_tensor("tti", [S, 1], I32, kind="ExternalInput")
    d_mask = nc.dram_tensor("maskadd", [S], F32, kind="ExternalInput")
    d_wemb = nc.dram_tensor("wemb", [V, D], F32, kind="ExternalInput")
    d_pemb = nc.dram_tensor("pemb", [S, D], F32, kind="ExternalInput")
    d_temb = nc.dram_tensor("temb", [2, D], F32, kind="ExternalInput")
    d_eg = nc.dram_tensor("eg", [D], F32, kind="ExternalInput")
    d_eb = nc.dram_tensor("eb", [D], F32, kind="ExternalInput")
    # pre-tiled weights: each tile is one contiguous-per-partition DMA
    d_wq = nc.dram_tensor("wq", [L, NT, 128, NT, 128], BF16, kind="ExternalInput")
    d_wk = nc.dram_tensor("wk", [L, NT, 128, NT, 128], BF16, kind="ExternalInput")
    d_wv = nc.dram_tensor("wv", [L, 3, 128, NT, 256], BF16, kind="ExternalInput")
    d_wo = nc.dram_tensor("wo", [L, NT, 128, NT, 128], BF16, kind="ExternalInput")
    d_w1 = nc.dram_tensor("w1", [L, NTF // 2, 128, NT, 256], BF16, kind="ExternalInput")
    d_w2 = nc.dram_tensor("w2", [L, NTF // 2, 128, 2, D], BF16, kind="ExternalInput")
    d_bq = nc.dram_tensor("bq", [L, D], F32, kind="ExternalInput")
    d_bk = nc.dram_tensor("bk", [L, D], F32, kind="ExternalInput")
    d_bv = nc.dram_tensor("bv", [L, D], F32, kind="ExternalInput")
    d_bo = nc.dram_tensor("bo", [L, D], F32, kind="ExternalInput")
    d_b2 = nc.dram_tensor("b2", [L, D], F32, kind="ExternalInput")  # b2 + W2@b1'
    d_fg = nc.dram_tensor("fg", [L, D], F32, kind="ExternalInput")
    d_fb = nc.dram_tensor("fb", [L, D], F32, kind="ExternalInput")
    d_ones = nc.dram_tensor("ones128", [128], F32, kind="ExternalInput")
    d_onesb = nc.dram_tensor("ones128b", [128], BF16, kind="ExternalInput")
    d_ones512 = nc.dram_tensor("ones512", [1, 512], F32, kind="ExternalInput")
    d_neg1 = nc.dram_tensor("neg1", [1, 128], F32, kind="ExternalInput")
    d_sel2 = nc.dram_tensor("sel2", [33, 128], F32, kind="ExternalInput")
    d_w1rs = nc.dram_tensor("w1rs", [L, F], BF16, kind="ExternalInput")
    d_onesgb = nc.dram_tensor("onesgridb", [128, NST * H], BF16, kind="ExternalInput")
    d_out = nc.dram_tensor("out", [S, D], F32, kind="ExternalOutput")

    with tile.TileContext(nc) as tc:
        _emit(nc, tc, locals())
    _legalize_waits(nc)
    return nc


def _emit(nc, tc, d):
    import contextlib
    ctx = contextlib.ExitStack()
    with ctx:
        _emit_body(nc, tc, d, ctx)


def _emit_body(nc, tc, d, ctx):
    pool = ctx.enter_context(tc.tile_pool(name="persist", bufs=1))
    wqkpool = ctx.enter_context(tc.tile_pool(name="wqk", bufs=6))
    wvpool = ctx.enter_context(tc.tile_pool(name="wv", bufs=3))
    wopool = ctx.enter_context(tc.tile_pool(name="wo", bufs=3))
    w1pool = ctx.enter_context(tc.tile_pool(name="w1", bufs=4))
    w2pool = ctx.enter_context(tc.tile_pool(name="w2", bufs=4))
    ppool = ctx.enter_context(tc.tile_pool(name="params", bufs=2))
    epool = ctx.enter_context(tc.tile_pool(name="epool", bufs=16))
    hpool = ctx.enter_context(tc.tile_pool(name="hpool", bufs=6))
    spool = ctx.enter_context(tc.tile_pool(name="smalls", bufs=1))
    w1rpool = ctx.enter_context(tc.tile_pool(name="w1r", bufs=1))
    # one psum pool per bank-tag group (8 banks total, managed by tag):
    # ps_a: tags a0,a1 (2 banks) - qkv/v/wo acc ping-pong, ffn hacc, ln2 stats
    # ps_s: tag s_rot (3 banks) - score tiles, ln bcasts, ffn y0-y2
    # ps_c: tags c0,c1 (2 banks) - attention ctx, ln1 bc1, ffn y3-y4
    # ps_r: tag r0 (1 bank)     - recip bcast, ffn y5
    ps_a = ctx.enter_context(tc.tile_pool(name="ps_a", bufs=1, space="PSUM"))
    ps_s = ctx.enter_context(tc.tile_pool(name="ps_s", bufs=3, space="PSUM"))
    ps_c = ctx.enter_context(tc.tile_pool(name="ps_c", bufs=1, space="PSUM"))
    ps_r = ctx.enter_context(tc.tile_pool(name="ps_r", bufs=1, space="PSUM"))

    # ---- persistent constants --------------------------------------------
    ones_col = pool.tile([128, 1], F32R, name="ones_col")
    nc.sync.dma_start(ones_col[:], d["d_ones"].ap().rearrange("(p o) -> p o", o=1).bitcast(F32R))
    ones_colb = pool.tile([128, 1], BF16, name="ones_colb")
    nc.sync.dma_start(ones_colb[:], d["d_onesb"].ap().rearrange("(p o) -> p o", o=1))
    one_row = pool.tile([1, 128], F32R, name="one_row")
    nc.sync.dma_start(one_row[:], d["d_ones"].ap().rearrange("(o p) -> o p", o=1).bitcast(F32R))
    ones_s = pool.tile([1, 512], F32R, name="ones_s")
    nc.sync.dma_start(ones_s[:], d["d_ones512"].ap()[:, :].bitcast(F32R))
    neg_row = pool.tile([1, 128], F32R, name="neg_row")
    nc.sync.dma_start(neg_row[:], d["d_neg1"].ap()[:, :].bitcast(F32R))
    sel2 = pool.tile([33, 128], F32R, name="sel2")
    nc.sync.dma_start(sel2[:], d["d_sel2"].ap()[:, :].bitcast(F32R))
    r01 = pool.tile([33, S], F32R, name="r01")
    nc.vector.memset(r01[:].bitcast(F32), 0.0)
    dpair = pool.tile([33, S], F32, name="dpair")
    nc.vector.memset(dpair[:], 1.0)
    ident = pool.tile([128, 128], F32, name="ident")
    make_identity(nc, ident[:])
    ident16 = pool.tile([128, 128], BF16, name="ident16")
    make_identity(nc, ident16[:])
    maskc = pool.tile([128, NST], F32, name="maskc")
    nc.sync.dma_start(maskc[:], d["d_mask"].ap().rearrange("(n p) -> p n", p=128))

    # ---- persistent activations ------------------------------------------
    xT = pool.tile([128, NT, S], BF16, name="xT")       # layer input, feature-major
    aT = pool.tile([128, NT, S], BF16, name="aT")       # post-attn LN out
    qT = pool.tile([128, NT, S], BF16, name="qT")
    kT = pool.tile([128, NT, S], BF16, name="kT")
    cT = pool.tile([128, NT, S], BF16, name="cT")       # ctx, feature-major
    ybuf = pool.tile([128, NT, S], F32R, name="ybuf")   # pre-LN staging
    vaug = pool.tile([128, NST, H, DK + 1], BF16, name="vaug")
    # ones column of vaug (written once)
    nc.sync.dma_start(
        vaug[:, :, :, DK:DK + 1],
        d["d_onesgb"].ap().rearrange("p (a b) -> p a b", a=NST)[:, :, :],
    )

    # =======================================================================
    # layernorm along the feature (partition-spread) dim, feature-major.
    # y: [128, nt, S] F32R tile; writes out[:, dt, :].
    # st0/st1/bc0/bc1: psum tiles supplied by the caller (bank-tag plan).
    # If g_col is None the gamma/beta were folded into the next matmul.
    def layernorm(y, nt, eps, out, dim, st0, st1, bc0, bc1,
                  g_col=None, b_col=None):
        sq_t = []
        for dt in range(nt):
            sqt = spool.tile([128, S], BF16, name=f"sq{dt}", tag=f"sq{dt % 2}")
            nc.scalar.activation(sqt[:], y[:, dt, :].bitcast(F32), AF.Square)
            sq_t.append(sqt)
        for dt in range(nt):
            nc.tensor.matmul(st0[:], ones_col[:], y[:, dt, :],
                             start=(dt == 0), stop=(dt == nt - 1))
        for dt in range(nt):
            nc.tensor.matmul(st1[:], ones_colb[:], sq_t[dt][:],
                             start=(dt == 0), stop=(dt == nt - 1))
        mu = spool.tile([1, S], F32R, name="mu", tag="ln_mu")
        nc.vector.tensor_scalar(mu[:], st0[:], 1.0 / dim, None, OP.mult)
        # negmu broadcast doubles as a PE warm-keeper early in the chain
        nc.tensor.matmul(bc0[:], neg_row[:], mu[:], start=True, stop=True)
        msq = spool.tile([1, S], F32R, name="msq", tag="ln_msq")
        nc.vector.tensor_scalar(msq[:], st1[:], 1.0 / dim, eps, OP.mult, OP.add)
        nc.tensor.matmul(bc1[:], one_row[:], msq[:], start=True, stop=True)
        musq = spool.tile([1, S], F32R, name="musq", tag="ln_musq")
        nc.vector.tensor_tensor(musq[:], mu[:].bitcast(F32), mu[:].bitcast(F32), op=OP.mult)
        nc.tensor.matmul(bc1[:], one_row[:], musq[:], start=True, stop=True)
        var = spool.tile([1, S], F32R, name="var", tag="ln_var")
        nc.vector.tensor_tensor(var[:], msq[:].bitcast(F32), musq[:].bitcast(F32), op=OP.subtract)
        # warm-keeper mid-chain (overwritten by rstd_ps below)
        nc.tensor.matmul(bc1[:], one_row[:], var[:], start=True, stop=True)
        lnv = spool.tile([1, S], F32R, name="lnv", tag="ln_lnv")
        nc.scalar.activation(lnv[:], var[:].bitcast(F32), AF.Ln)
        nc.tensor.matmul(bc1[:], one_row[:], lnv[:], start=True, stop=True)
        rstd = spool.tile([1, S], F32R, name="rstd", tag="ln_rstd")
        nc.scalar.activation(rstd[:], lnv[:].bitcast(F32), AF.Exp, scale=-0.5)
        nc.tensor.matmul(bc1[:], one_row[:], rstd[:], start=True, stop=True)
        # pass 1 (in-place): y -= mu
        for dt in range(nt):
            nc.vector.tensor_tensor(y[:, dt, :], y[:, dt, :].bitcast(F32),
                                    bc0[:], op=OP.add)
            if g_col is None:
                nc.vector.tensor_tensor(out[:, dt, :], y[:, dt, :].bitcast(F32),
                                        bc1[:], op=OP.mult)
            else:
                nc.vector.tensor_tensor(out[:, dt, :], y[:, dt, :].bitcast(F32),
                                        bc1[:], op=OP.mult)
                nc.vector.tensor_scalar(out[:, dt, :], out[:, dt, :],
                                        g_col[:, dt:dt + 1], b_col[:, dt:dt + 1],
                                        OP.mult, OP.add)

    # =======================================================================
    # embedding: gather + add + transpose to feature-major + LN -> xT
    with tc.tile_pool(name="emb_sb", bufs=4) as embp:
        egc = ppool.tile([128, NT], F32, name="egc")
        nc.sync.dma_start(egc[:], d["d_eg"].ap().rearrange("(n p) -> p n", p=128))
        ebc = ppool.tile([128, NT], F32, name="ebc")
        nc.sync.dma_start(ebc[:], d["d_eb"].ap().rearrange("(n p) -> p n", p=128))
        for st in range(NST):
            idst = embp.tile([128, 1], I32, name="idst", tag="idst")
            nc.sync.dma_start(idst[:], d["d_ids"].ap()[st * 128:(st + 1) * 128, :])
            ttst = embp.tile([128, 1], I32, name="ttst", tag="ttst")
            nc.sync.dma_start(ttst[:], d["d_tti"].ap()[st * 128:(st + 1) * 128, :])
            x0 = embp.tile([128, D], F32, name="x0", tag="x0")
            nc.gpsimd.indirect_dma_start(
                out=x0[:], out_offset=None, in_=d["d_wemb"].ap(),
                in_offset=bass.IndirectOffsetOnAxis(ap=idst[:, :1], axis=0))
            tg = embp.tile([128, D], F32, name="tg", tag="tg")
            nc.gpsimd.indirect_dma_start(
                out=tg[:], out_offset=None, in_=d["d_temb"].ap(),
                in_offset=bass.IndirectOffsetOnAxis(ap=ttst[:, :1], axis=0))
            pg = embp.tile([128, D], F32, name="pg", tag="pg")
            nc.sync.dma_start(pg[:], d["d_pemb"].ap()[st * 128:(st + 1) * 128, :])
            nc.vector.tensor_tensor(x0[:], x0[:], tg[:], op=OP.add)
            nc.vector.tensor_tensor(x0[:], x0[:], pg[:], op=OP.add)
            for dt in range(NT):
                trp = ps_s.tile([128, 128], F32, name="trp", tag="s_rot")
                nc.tensor.transpose(trp[:], x0[:, dt * 128:(dt + 1) * 128], ident[:])
                nc.vector.tensor_copy(ybuf[:, dt, st * 128:(st + 1) * 128], trp[:])
        est0 = ps_a.tile([1, S], F32, name="est0", tag="a0")
        est1 = ps_a.tile([1, S], F32, name="est1", tag="a1")
        ebc0 = ps_s.tile([128, S], F32, name="ebc0", tag="s_rot")
        ebc1 = ps_s.tile([128, S], F32, name="ebc1", tag="s_rot")
        layernorm(ybuf, NT, 1e-12, xT, D, est0, est1, ebc0, ebc1,
                  g_col=egc[:], b_col=ebc[:])

    # =======================================================================
    # transformer layers
    for l in range(L):
        # ---- per-layer params -------------------------------------------
        bqc = ppool.tile([128, NT], F32, name="bqc", tag="bqc")
        nc.sync.dma_start(bqc[:], d["d_bq"].ap()[l].rearrange("(n p) -> p n", p=128))
        bkc = ppool.tile([128, NT], F32, name="bkc", tag="bkc")
        nc.sync.dma_start(bkc[:], d["d_bk"].ap()[l].rearrange("(n p) -> p n", p=128))
        bvr = ppool.tile([1, D], F32R, name="bvr", tag="bvr")
        nc.sync.dma_start(bvr[:], d["d_bv"].ap()[l].rearrange("(o e) -> o e", o=1).bitcast(F32R))
        boc = ppool.tile([128, NT], F32, name="boc", tag="boc")
        nc.sync.dma_start(boc[:], d["d_bo"].ap()[l].rearrange("(n p) -> p n", p=128))
        b2c = ppool.tile([128, NT], F32, name="b2c", tag="b2c")
        nc.sync.dma_start(b2c[:], d["d_b2"].ap()[l].rearrange("(n p) -> p n", p=128))
        fgc = ppool.tile([128, NT], F32, name="fgc", tag="fgc")
        nc.sync.dma_start(fgc[:], d["d_fg"].ap()[l].rearrange("(n p) -> p n", p=128))
        fbc = ppool.tile([128, NT], F32, name="fbc", tag="fbc")
        nc.sync.dma_start(fbc[:], d["d_fb"].ap()[l].rearrange("(n p) -> p n", p=128))
        w1r = w1rpool.tile([1, F], BF16, name="w1r", tag="w1r")
        nc.sync.dma_start(w1r[:], d["d_w1rs"].ap()[l].rearrange("(o f) -> o f", o=1))

        # ---- phase 1: interleaved QKV + attention + Wo -------------------
        e_tiles = [None] * NP

        def emit_v(c):
            wv = wvpool.tile([128, NT, 256], BF16, name=f"wv{c}", tag="wv")
            nc.sync.dma_start(wv[:], d["d_wv"].ap()[l, c])
            for st in range(NST):
                acc = ps_a.tile([128, 256], F32, name=f"v{st}", tag=f"a{st % 2}")
                for dt in range(NT):
                    nc.tensor.matmul(acc[:], xT[:, dt, st * 128:(st + 1) * 128],
                                     wv[:, dt, :],
                                     start=(dt == 0), stop=False)
                nc.tensor.matmul(acc[:], one_row[:],
                                 bvr[:, c * 256:(c + 1) * 256],
                                 start=False, stop=True, skip_group_check=True)
                nc.vector.tensor_copy(
                    vaug[:, st, c * 4:(c + 1) * 4, 0:DK],
                    acc[:].rearrange("p (a b) -> p a b", a=4))

        def emit_qk(et):
            for (wd, bcol, dst, nm) in ((d["d_wq"], bqc, qT, "q"),
                                        (d["d_wk"], bkc, kT, "k")):
                wt = wqkpool.tile([128, NT, 128], BF16, name=f"w{nm}{et}", tag="wqk")
                nc.sync.dma_start(wt[:], wd.ap()[l, et])
                acc = ps_a.tile([128, S], F32, name=f"{nm}{et}", tag=f"a{et % 2}")
                for dt in range(NT):
                    nc.tensor.matmul(acc[:], wt[:, dt, :], xT[:, dt, :],
                                     start=(dt == 0), stop=(dt == NT - 1))
                nc.vector.tensor_scalar(dst[:, et, :], acc[:],
                                        bcol[:, et:et + 1], None, OP.add)

        def emit_scores(p):
            ets = []
            for kt in range(NST):
                scs = []
                for hh in range(2):
                    lo = hh * 64
                    sc = ps_s.tile([128, S], F32, name=f"sc{kt}{hh}", tag="s_rot")
                    nc.tensor.matmul(
                        sc[:], kT[lo:lo + 64, p, kt * 128:(kt + 1) * 128],
                        qT[lo:lo + 64, p, :], start=True, stop=True,
                        tile_position=(lo, 0))
                    scs.append(sc)
                for hh in range(2):
                    et = epool.tile([128, S], BF16, name=f"e{kt}{hh}", tag="e")
                    nc.scalar.activation(et[:], scs[hh][:], AF.Exp,
                                         bias=maskc[:, kt:kt + 1])
                    ets.append(et)
            e_tiles[p] = ets

        def emit_pv(p):
            ets = e_tiles[p]
            cpss = []
            for hh in range(2):
                h = 2 * p + hh
                cps = ps_c.tile([DK + 1, S], F32, name=f"cps{hh}", tag=f"c{hh}")
                for kt in range(NST):
                    nc.tensor.matmul(cps[:],
                                     vaug[:, kt, h, 0:DK + 1],
                                     ets[kt * 2 + hh][:],
                                     start=(kt == 0), stop=(kt == NST - 1))
                cpss.append(cps)
            nc.vector.tensor_copy(dpair[0:1, :], cpss[0][DK:DK + 1, :])
            nc.vector.tensor_copy(dpair[32:33, :], cpss[1][DK:DK + 1, :])
            nl33 = spool.tile([33, S], F32, name="nl33", tag="nl33")
            nc.scalar.activation(nl33[:], dpair[:], AF.Ln)
            nc.scalar.activation(r01[:], nl33[:], AF.Exp, scale=-1.0)
            rps = ps_r.tile([128, S], F32, name="rps", tag="r0")
            nc.tensor.matmul(rps[:], sel2[:], r01[:], start=True, stop=True)
            rsb = spool.tile([128, S], F32, name="rsb", tag=f"rsb{p % 2}")
            nc.vector.tensor_copy(rsb[:], rps[:])
            for hh in range(2):
                lo = hh * 64
                nc.vector.tensor_tensor(cT[lo:lo + DK, p, :], cpss[hh][0:DK, :],
                                        rsb[lo:lo + DK, :], op=OP.mult)

        def emit_wo(et, st0, st1):
            wt = wopool.tile([128, NT, 128], BF16, name=f"wo{et}", tag="wo")
            nc.sync.dma_start(wt[:], d["d_wo"].ap()[l, et])
            acc = ps_a.tile([128, S], F32, name=f"o{et}", tag=f"a{et % 2}")
            for dt in range(NT):
                nc.tensor.matmul(acc[:], wt[:, dt, :], cT[:, dt, :],
                                 start=(dt == 0), stop=(dt == NT - 1))
            nc.vector.scalar_tensor_tensor(
                ybuf[:, et, :], acc[:], boc[:, et:et + 1],
                xT[:, et, :], op0=OP.add, op1=OP.add)
            nc.vector.tensor_copy(aT[:, et, :], ybuf[:, et, :].bitcast(F32))
            # LN1 stats interleaved: square + partial sums as tiles land
            sqt = spool.tile([128, S], BF16, name=f"sq{et}", tag=f"sq{et % 2}")
            nc.scalar.activation(sqt[:], ybuf[:, et, :].bitcast(F32), AF.Square)
            nc.tensor.matmul(st0[:], ones_col[:], ybuf[:, et, :],
                             start=(et == 0), stop=(et == NT - 1),
                             skip_group_check=True)
            nc.tensor.matmul(st1[:], ones_colb[:], sqt[:],
                             start=(et == 0), stop=(et == NT - 1),
                             skip_group_check=True)

        # interleaved emission
        emit_v(0)
        emit_qk(0)
        emit_qk(1)
        emit_scores(0)
        emit_v(1)
        emit_qk(2)
        emit_scores(1)
        emit_pv(0)
        emit_v(2)
        emit_qk(3)
        emit_scores(2)
        emit_pv(1)
        emit_qk(4)
        emit_scores(3)
        emit_pv(2)
        emit_qk(5)
        emit_scores(4)
        emit_pv(3)
        emit_scores(5)
        emit_pv(4)
        emit_pv(5)

        # ---- Wo + residual + LN1 (gamma/beta folded into W1) -------------
        lst0 = ps_s.tile([1, S], F32, name="lst0", tag="s_rot")
        lst1 = ps_s.tile([1, S], F32, name="lst1", tag="s_rot")
        for et in range(NT):
            emit_wo(et, lst0, lst1)
        lbc1 = ps_c.tile([128, S], F32, name="lbc1", tag="c0")
        # LN1 chain: apply is folded into FFN (rank-1 mu correction via W1
        # row-sums + rstd applied at the hs cast), so W1 never waits on it.
        mu = spool.tile([1, S], F32R, name="mu", tag="ln_mu")
        nc.vector.tensor_scalar(mu[:], lst0[:], 1.0 / D, None, OP.mult)
        negmub = spool.tile([1, S], BF16, name="negmub", tag="ln_negmub")
        nc.vector.tensor_scalar(negmub[:], mu[:].bitcast(F32), -1.0, None, OP.mult)
        msq = spool.tile([1, S], F32R, name="msq", tag="ln_msq")
        nc.vector.tensor_scalar(msq[:], lst1[:], 1.0 / D, 1e-5, OP.mult, OP.add)
        nc.tensor.matmul(lbc1[:], one_row[:], msq[:], start=True, stop=True)
        musq = spool.tile([1, S], F32R, name="musq", tag="ln_musq")
        nc.vector.tensor_tensor(musq[:], mu[:].bitcast(F32), mu[:].bitcast(F32), op=OP.mult)
        nc.tensor.matmul(lbc1[:], one_row[:], musq[:], start=True, stop=True)
        var = spool.tile([1, S], F32R, name="var", tag="ln_var")
        nc.vector.tensor_tensor(var[:], msq[:].bitcast(F32), musq[:].bitcast(F32), op=OP.subtract)
        nc.tensor.matmul(lbc1[:], one_row[:], var[:], start=True, stop=True)
        lnv = spool.tile([1, S], F32R, name="lnv", tag="ln_lnv")
        nc.scalar.activation(lnv[:], var[:].bitcast(F32), AF.Ln)
        nc.tensor.matmul(lbc1[:], one_row[:], lnv[:], start=True, stop=True)
        rstd = spool.tile([1, S], F32R, name="rstd", tag="ln_rstd")
        nc.scalar.activation(rstd[:], lnv[:].bitcast(F32), AF.Exp, scale=-0.5)
        nc.tensor.matmul(lbc1[:], one_row[:], rstd[:], start=True, stop=True)
        rstd_sb = spool.tile([128, S], F32, name="rstd_sb", tag="rstd_sb")
        nc.vector.tensor_copy(rstd_sb[:], lbc1[:])

        # ---- phase 4: FFN (W1 -> h, W2 accumulate into 6 yT banks) -------
        YT = ["s_rot", "s_rot", "s_rot", "c0", "c1", "r0"]
        ytiles = []
        for et in range(NT):
            yt = (ps_s if YT[et].startswith("s") else (ps_c if YT[et].startswith("c") else ps_r)
                  ).tile([128, S], F32, name=f"yt{et}", tag=YT[et])
            ytiles.append(yt)
        h_sb = [None] * NTF
        w2ts = [None] * NTF

        def emit_h(f):
            c, fj = divmod(f, 2)
            if fj == 0:
                w1t = w1pool.tile([128, NT, 256], BF16, name=f"w1_{c}", tag="w1")
                nc.sync.dma_start(w1t[:], d["d_w1"].ap()[l, c])
                emit_h.w1t = w1t
                w2t = w2pool.tile([128, 2, D], BF16, name=f"w2_{c}", tag="w2")
                nc.sync.dma_start(w2t[:], d["d_w2"].ap()[l, c])
                emit_h.w2t = w2t
            hacc = ps_a.tile([128, S], F32, name=f"h{f}", tag=f"a{f % 2}")
            for dt in range(NT):
                nc.tensor.matmul(hacc[:], emit_h.w1t[:, dt, fj * 128:(fj + 1) * 128],
                                 aT[:, dt, :], start=(dt == 0), stop=False)
            nc.tensor.matmul(hacc[:], w1r[0:1, f * 128:(f + 1) * 128],
                             negmub[:], start=False, stop=True,
                             skip_group_check=True)
            hs = hpool.tile([128, S], BF16, name=f"hs{f}", tag="hs")
            nc.vector.tensor_tensor(hs[:], hacc[:], rstd_sb[:], op=OP.mult)
            h_sb[f] = hs
            w2ts[f] = emit_h.w2t

        def emit_y(f):
            fj = f % 2
            w2t = w2ts[f]
            for et in range(NT):
                nc.tensor.matmul(ytiles[et][:],
                                 w2t[:, fj, et * 128:(et + 1) * 128],
                                 h_sb[f][:], start=(f == 0), stop=(f == NTF - 1),
                                 skip_group_check=True)
            h_sb[f] = None

        emit_h(0)
        for f in range(1, NTF):
            emit_h(f)
            emit_y(f - 1)
        emit_y(NTF - 1)

        # epilogue: +b2'' (per-partition) -> ybuf
        for et in range(NT):
            nc.vector.tensor_scalar(ybuf[:, et, :], ytiles[et][:],
                                    b2c[:, et:et + 1], None, OP.add)

        # ---- LN2 -> xT (next layer input) --------------------------------
        nst0 = ps_a.tile([1, S], F32, name="nst0", tag="a0")
        nst1 = ps_a.tile([1, S], F32, name="nst1", tag="a1")
        nbc0 = ps_s.tile([128, S], F32, name="nbc0", tag="s_rot")
        nbc1 = ps_s.tile([128, S], F32, name="nbc1", tag="s_rot")
        layernorm(ybuf, NT, 1e-5, xT, D, nst0, nst1, nbc0, nbc1,
                  g_col=fgc[:], b_col=fbc[:])

    # =======================================================================
    # output: transpose xT -> [S, D] and DMA out
    with tc.tile_pool(name="out_sb", bufs=2) as outp:
        for st in range(NST):
            ops_t = ps_s.tile([128, D], BF16, name="ops", tag="s_rot")
            for dt in range(NT):
                nc.tensor.transpose(ops_t[:, dt * 128:(dt + 1) * 128],
                                    xT[:, dt, st * 128:(st + 1) * 128],
                                    ident16[:])
            osb = outp.tile([128, D], F32, name="osb", tag="osb")
            nc.vector.tensor_copy(osb[:], ops_t[:])
            nc.sync.dma_start(d["d_out"].ap()[st * 128:(st + 1) * 128, :], osb[:])


# ---------------------------------------------------------------------------
def _sel2():
    s = np.zeros((33, 128), np.float32)
    s[0, 0:64] = 1.0
    s[32, 64:128] = 1.0
    return s


def kernel(**inputs):
    global _NC_CACHE
    if _NC_CACHE is None:
        _NC_CACHE = _build_nc()
    nc = _NC_CACHE

    import ml_dtypes
    f32 = lambda a: np.ascontiguousarray(np.asarray(a), dtype=np.float32)
    bf = lambda a: np.ascontiguousarray(a.astype(ml_dtypes.bfloat16))

    Wq = f32(inputs["Wq"]) * SCALE
    bq = f32(inputs["bq"]) * SCALE
    Wk, Wv, Wo = f32(inputs["Wk"]), f32(inputs["Wv"]), f32(inputs["Wo"])
    W1, W2 = f32(inputs["W1"]), f32(inputs["W2"])
    ag, ab = f32(inputs["attn_ln_g"]), f32(inputs["attn_ln_b"])
    b1, b2 = f32(inputs["b1"]), f32(inputs["b2"])
    # fold LN1 gamma into W1 columns, beta into b1; then b1 into b2
    W1f = W1 * ag[:, None, :]                    # [L,F,D] * [L,1,D]
    b1f = b1 + np.einsum("lfd,ld->lf", W1, ab)
    b2f = b2 + np.einsum("ldf,lf->ld", W2, b1f)

    def tile_qk(W):  # [L, Dout, Din] -> [L, et, p, n, e]
        WT = W.transpose(0, 2, 1)                # [L, Din, Dout]
        return bf(WT.reshape(L, NT, 128, NT, 128).transpose(0, 3, 2, 1, 4))

    def tile_v(W):   # -> [L, c3, p, n, e256]
        WT = W.transpose(0, 2, 1)
        return bf(WT.reshape(L, NT, 128, 3, 256).transpose(0, 3, 2, 1, 4))

    def tile_w1(W):  # [L, F, D] -> [L, c12, p, n6, e256]
        WT = W.transpose(0, 2, 1)                # [L, D, F]
        return bf(WT.reshape(L, NT, 128, NTF // 2, 256).transpose(0, 3, 2, 1, 4))

    def tile_w2(W):  # [L, D, F] -> [L, c12, p, g2, e768]
        WT = W.transpose(0, 2, 1)                # [L, F, D]
        return bf(WT.reshape(L, NTF // 2, 2, 128, D).transpose(0, 1, 3, 2, 4))

    shared = {
        "wemb": f32(inputs["word_emb"]),
        "pemb": f32(inputs["pos_emb"])[:S],
        "temb": f32(inputs["type_emb"]),
        "eg": f32(inputs["emb_ln_g"]), "eb": f32(inputs["emb_ln_b"]),
        "wq": tile_qk(Wq),
        "wk": tile_qk(Wk),
        "wv": tile_v(Wv),
        "wo": tile_qk(Wo),
        "w1": tile_w1(W1f),
        "w1rs": bf(W1f.sum(axis=2)),
        "w2": tile_w2(W2),
        "bq": bq, "bk": f32(inputs["bk"]), "bv": f32(inputs["bv"]),
        "bo": f32(inputs["bo"]), "b2": b2f,
        "fg": f32(inputs["ffn_ln_g"]), "fb": f32(inputs["ffn_ln_b"]),
        "ones128": np.ones(128, np.float32),
        "ones128b": np.ones(128, ml_dtypes.bfloat16),
        "ones512": np.ones((1, 512), np.float32),
        "neg1": np.full((1, 128), -1.0, np.float32),
        "sel2": _sel2(),
        "onesgridb": np.ones((128, NST * H), ml_dtypes.bfloat16),
    }
    ids = np.asarray(inputs["input_ids"]).astype(np.int32)
    tti = np.asarray(inputs["token_type_ids"]).astype(np.int32)
    am = np.asarray(inputs["attention_mask"]).astype(np.float32)
    in_maps = []
    for c in range(B):
        in_maps.append({
            **shared,
            "ids": ids[c].reshape(S, 1),
            "tti": tti[c].reshape(S, 1),
            "maskadd": np.where(am[c] == 0, -1e9, 0.0).astype(np.float32),
        })
    res = bass_utils.run_bass_kernel_spmd(
        nc, in_maps, core_ids=list(range(B)), trace=False)
    out = np.stack([res.results[c]["out"] for c in range(B)], axis=0)
    return out.astype(np.float32)


# revision 29
# speedup vs baseline: 1.0283x; 1.0283x over previous
"""BERT-base (12-layer, B=8, S=512, D=768, H=12, F=3072) forward pass on 8
Trainium2 NeuronCores.

Strategy: data-parallel over batch (1 sequence per core, no collectives).
Per core, activations are kept feature-major xT[D, S] in SBUF so that:
  - all big matmuls run as lhsT[dK,128] @ rhs[dK,512] at full PE rate
  - LayerNorm statistics (sums over the feature dim = partition dim) are
    ones-vector matmuls on the PE
  - softmax runs in scoresT [k, q] layout: the attention mask is a
    per-partition exp() bias, and denominators fall out of a ones-augmented
    V column in the P@V matmul (M=65)
  - per-column broadcasts (mu, rstd, 1/denom) are rank-1 ones-matmuls
v2 changes vs baseline:
  - QKV + attention + Wo emission interleaved so ACT exp overlaps PE work
    (keeps the PE dense -> HAM stays at full clock)
  - denominator folded into PV (ones-augmented V column) - kills 24
    ones-matmuls/layer
  - score matmuls (K=64) packed pairwise into disjoint PE row groups via
    tile_position -> 2x concurrency
  - host-side weight relayout: every weight tile is ONE dma_start with
    1.5-3KB contiguous per-partition lines
  - LN1 gamma/beta folded into W1 (host), b1 folded into b2 (host)
  - exp-tile pool 16-deep; LN scalar chains bridged with warm matmuls
  - deterministic 8-bank PSUM tag plan
"""
import numpy as np

import concourse.bass as bass
import concourse.mybir as mybir
import concourse.tile as tile
from concourse import bass_utils
from concourse.masks import make_identity

AF = mybir.ActivationFunctionType
OP = mybir.AluOpType
F32 = mybir.dt.float32
F32R = mybir.dt.float32r
BF16 = mybir.dt.bfloat16
I32 = mybir.dt.int32

B, S, D, H, F, L, V = 8, 512, 768, 12, 3072, 12, 30522
DK = D // H
SCALE = 1.0 / float(np.sqrt(DK))
NT = D // 128      # 6 feature tiles
NTF = F // 128     # 24 ffn tiles
NST = S // 128     # 4 sequence tiles
NP = H // 2        # 6 head pairs

_NC_CACHE = None


# ---------------------------------------------------------------------------
# wait-slot legalization: walrus codegen allows only ONE sync-wait command on
# TPB instructions; hoist excess waits into standalone EventSemaphores.
def _legalize_waits(nc):
    skip = (mybir.InstEventSemaphore, mybir.InstNoOp)
    n = 0
    for fn in nc.m.functions:
        for blk in fn.blocks:
            out = []
            for inst in blk.instructions:
                si = inst.sync_info
                if si is not None and si.on_wait and not isinstance(inst, skip) \
                        and len(si.on_wait) > 1:
                    waits = list(si.on_wait)
                    for j, w in enumerate(waits[:-1]):
                        ev = mybir.InstEventSemaphore(
                            name=f"{inst.name}-lgw{j}", ins=[], outs=[],
                            sync_info=mybir.SyncInfo(on_wait=[w], on_update=[]),
                        )
                        ev.engine = inst.engine
                        out.append(ev)
                        n += 1
                    inst.sync_info = mybir.SyncInfo(
                        on_wait=[waits[-1]], on_update=list(si.on_update))
                out.append(inst)
            try:
                blk.instructions = out
            except Exception:
                blk.instructions.clear()
                blk.instructions.extend(out)
    return n


def _build_nc():
    nc = bass.Bass("TRN2", target_bir_lowering=False, debug=False,
                   enable_asserts=False, num_devices=8)

    # ---- DRAM I/O ---------------------------------------------------------
    d_ids = nc.dram_tensor("ids", [S, 1], I32, kind="ExternalInput")
    d_tti = nc.dram_tensor("tti", [S, 1], I32, kind="ExternalInput")
    d_mask = nc.dram_tensor("maskadd", [S], F32, kind="ExternalInput")
    d_wemb = nc.dram_tensor("wemb", [V, D], F32, kind="ExternalInput")
    d_pemb = nc.dram_tensor("pemb", [S, D], F32, kind="ExternalInput")
    d_temb = nc.dram_tensor("temb", [2, D], F32, kind="ExternalInput")
    d_eg = nc.dram_tensor("eg", [D], F32, kind="ExternalInput")
    d_eb = nc.dram_tensor("eb", [D], F32, kind="ExternalInput")
    # pre-tiled weights: each tile is one contiguous-per-partition DMA
    d_wq = nc.dram_tensor("wq", [L, NT, 128, NT, 128], BF16, kind="ExternalInput")
    d_wk = nc.dram_tensor("wk", [L, NT, 128, NT, 128], BF16, kind="ExternalInput")
    d_wv = nc.dram_tensor("wv", [L, 3, 128, NT, 256], BF16, kind="ExternalInput")
    d_wo = nc.dram_tensor("wo", [L, NT, 128, NT, 128], BF16, kind="ExternalInput")
    d_w1 = nc.dram_tensor("w1", [L, NTF // 2, 128, NT, 256], BF16, kind="ExternalInput")
    d_w2 = nc.dram_tensor("w2", [L, NTF // 2, 128, 2, D], BF16, kind="ExternalInput")
    d_bq = nc.dram_tensor("bq", [L, D], F32, kind="ExternalInput")
    d_bk = nc.dram_tensor("bk", [L, D], F32, kind="ExternalInput")
    d_bv = nc.dram_tensor("bv", [L, D], F32, kind="ExternalInput")
    d_bo = nc.dram_tensor("bo", [L, D], F32, kind="ExternalInput")
    d_b2 = nc.dram_tensor("b2", [L, D], F32, kind="ExternalInput")  # b2 + W2@b1'
    d_fg = nc.dram_tensor("fg", [L, D], F32, kind="ExternalInput")
    d_fb = nc.dram_tensor("fb", [L, D], F32, kind="ExternalInput")
    d_ones = nc.dram_tensor("ones128", [128], F32, kind="ExternalInput")
    d_onesb = nc.dram_tensor("ones128b", [128], BF16, kind="ExternalInput")
    d_ones512 = nc.dram_tensor("ones512", [1, 512], F32, kind="ExternalInput")
    d_neg1 = nc.dram_tensor("neg1", [1, 128], F32, kind="ExternalInput")
    d_sel2 = nc.dram_tensor("sel2", [33, 128], F32, kind="ExternalInput")
    d_onesgb = nc.dram_tensor("onesgridb", [128, NST * H], BF16, kind="ExternalInput")
    d_out = nc.dram_tensor("out", [S, D], F32, kind="ExternalOutput")

    with tile.TileContext(nc) as tc:
        _emit(nc, tc, locals())
    _legalize_waits(nc)
    return nc


def _emit(nc, tc, d):
    import contextlib
    ctx = contextlib.ExitStack()
    with ctx:
        _emit_body(nc, tc, d, ctx)


def _emit_body(nc, tc, d, ctx):
    pool = ctx.enter_context(tc.tile_pool(name="persist", bufs=1))
    wqkpool = ctx.enter_context(tc.tile_pool(name="wqk", bufs=6))
    wvpool = ctx.enter_context(tc.tile_pool(name="wv", bufs=3))
    wopool = ctx.enter_context(tc.tile_pool(name="wo", bufs=3))
    w1pool = ctx.enter_context(tc.tile_pool(name="w1", bufs=4))
    w2pool = ctx.enter_context(tc.tile_pool(name="w2", bufs=4))
    ppool = ctx.enter_context(tc.tile_pool(name="params", bufs=2))
    epool = ctx.enter_context(tc.tile_pool(name="epool", bufs=16))
    hpool = ctx.enter_context(tc.tile_pool(name="hpool", bufs=6))
    spool = ctx.enter_context(tc.tile_pool(name="smalls", bufs=1))
    # one psum pool per bank-tag group (8 banks total, managed by tag):
    # ps_a: tags a0,a1 (2 banks) - qkv/v/wo acc ping-pong, ffn hacc, ln2 stats
    # ps_s: tag s_rot (3 banks) - score tiles, ln bcasts, ffn y0-y2
    # ps_c: tags c0,c1 (2 banks) - attention ctx, ln1 bc1, ffn y3-y4
    # ps_r: tag r0 (1 bank)     - recip bcast, ffn y5
    ps_a = ctx.enter_context(tc.tile_pool(name="ps_a", bufs=1, space="PSUM"))
    ps_s = ctx.enter_context(tc.tile_pool(name="ps_s", bufs=3, space="PSUM"))
    ps_c = ctx.enter_context(tc.tile_pool(name="ps_c", bufs=1, space="PSUM"))
    ps_r = ctx.enter_context(tc.tile_pool(name="ps_r", bufs=1, space="PSUM"))

    # ---- persistent constants --------------------------------------------
    ones_col = pool.tile([128, 1], F32R, name="ones_col")
    nc.sync.dma_start(ones_col[:], d["d_ones"].ap().rearrange("(p o) -> p o", o=1).bitcast(F32R))
    ones_colb = pool.tile([128, 1], BF16, name="ones_colb")
    nc.sync.dma_start(ones_colb[:], d["d_onesb"].ap().rearrange("(p o) -> p o", o=1))
    one_row = pool.tile([1, 128], F32R, name="one_row")
    nc.sync.dma_start(one_row[:], d["d_ones"].ap().rearrange("(o p) -> o p", o=1).bitcast(F32R))
    ones_s = pool.tile([1, 512], F32R, name="ones_s")
    nc.sync.dma_start(ones_s[:], d["d_ones512"].ap()[:, :].bitcast(F32R))
    neg_row = pool.tile([1, 128], F32R, name="neg_row")
    nc.sync.dma_start(neg_row[:], d["d_neg1"].ap()[:, :].bitcast(F32R))
    sel2 = pool.tile([33, 128], F32R, name="sel2")
    nc.sync.dma_start(sel2[:], d["d_sel2"].ap()[:, :].bitcast(F32R))
    r01 = pool.tile([33, S], F32R, name="r01")
    nc.vector.memset(r01[:].bitcast(F32), 0.0)
    dpair = pool.tile([33, S], F32, name="dpair")
    nc.vector.memset(dpair[:], 1.0)
    ident = pool.tile([128, 128], F32, name="ident")
    make_identity(nc, ident[:])
    ident16 = pool.tile([128, 128], BF16, name="ident16")
    make_identity(nc, ident16[:])
    maskc = pool.tile([128, NST], F32, name="maskc")
    nc.sync.dma_start(maskc[:], d["d_mask"].ap().rearrange("(n p) -> p n", p=128))

    # ---- persistent activations ------------------------------------------
    xT = pool.tile([128, NT, S], BF16, name="xT")       # layer input, feature-major
    aT = pool.tile([128, NT, S], BF16, name="aT")       # post-attn LN out
    qT = pool.tile([128, NT, S], BF16, name="qT")
    kT = pool.tile([128, NT, S], BF16, name="kT")
    cT = pool.tile([128, NT, S], BF16, name="cT")       # ctx, feature-major
    ybuf = pool.tile([128, NT, S], F32R, name="ybuf")   # pre-LN staging
    vaug = pool.tile([128, NST, H, DK + 1], BF16, name="vaug")
    # ones column of vaug (written once)
    nc.sync.dma_start(
        vaug[:, :, :, DK:DK + 1],
        d["d_onesgb"].ap().rearrange("p (a b) -> p a b", a=NST)[:, :, :],
    )

    # =======================================================================
    # layernorm along the feature (partition-spread) dim, feature-major.
    # y: [128, nt, S] F32R tile; writes out[:, dt, :].
    # st0/st1/bc0/bc1: psum tiles supplied by the caller (bank-tag plan).
    # If g_col is None the gamma/beta were folded into the next matmul.
    def layernorm(y, nt, eps, out, dim, st0, st1, bc0, bc1,
                  g_col=None, b_col=None):
        sq_t = []
        for dt in range(nt):
            sqt = spool.tile([128, S], BF16, name=f"sq{dt}", tag=f"sq{dt % 2}")
            nc.scalar.activation(sqt[:], y[:, dt, :].bitcast(F32), AF.Square)
            sq_t.append(sqt)
        for dt in range(nt):
            nc.tensor.matmul(st0[:], ones_col[:], y[:, dt, :],
                             start=(dt == 0), stop=(dt == nt - 1))
        for dt in range(nt):
            nc.tensor.matmul(st1[:], ones_colb[:], sq_t[dt][:],
                             start=(dt == 0), stop=(dt == nt - 1))
        mu = spool.tile([1, S], F32R, name="mu", tag="ln_mu")
        nc.vector.tensor_scalar(mu[:], st0[:], 1.0 / dim, None, OP.mult)
        # negmu broadcast doubles as a PE warm-keeper early in the chain
        nc.tensor.matmul(bc0[:], neg_row[:], mu[:], start=True, stop=True)
        msq = spool.tile([1, S], F32R, name="msq", tag="ln_msq")
        nc.vector.tensor_scalar(msq[:], st1[:], 1.0 / dim, eps, OP.mult, OP.add)
        nc.tensor.matmul(bc1[:], one_row[:], msq[:], start=True, stop=True)
        musq = spool.tile([1, S], F32R, name="musq", tag="ln_musq")
        nc.vector.tensor_tensor(musq[:], mu[:].bitcast(F32), mu[:].bitcast(F32), op=OP.mult)
        nc.tensor.matmul(bc1[:], one_row[:], musq[:], start=True, stop=True)
        var = spool.tile([1, S], F32R, name="var", tag="ln_var")
        nc.vector.tensor_tensor(var[:], msq[:].bitcast(F32), musq[:].bitcast(F32), op=OP.subtract)
        # warm-keeper mid-chain (overwritten by rstd_ps below)
        nc.tensor.matmul(bc1[:], one_row[:], var[:], start=True, stop=True)
        lnv = spool.tile([1, S], F32R, name="lnv", tag="ln_lnv")
        nc.scalar.activation(lnv[:], var[:].bitcast(F32), AF.Ln)
        nc.tensor.matmul(bc1[:], one_row[:], lnv[:], start=True, stop=True)
        rstd = spool.tile([1, S], F32R, name="rstd", tag="ln_rstd")
        nc.scalar.activation(rstd[:], lnv[:].bitcast(F32), AF.Exp, scale=-0.5)
        nc.tensor.matmul(bc1[:], one_row[:], rstd[:], start=True, stop=True)
        # pass 1 (in-place): y -= mu
        for dt in range(nt):
            nc.vector.tensor_tensor(y[:, dt, :], y[:, dt, :].bitcast(F32),
                                    bc0[:], op=OP.add)
            if g_col is None:
                nc.vector.tensor_tensor(out[:, dt, :], y[:, dt, :].bitcast(F32),
                                        bc1[:], op=OP.mult)
            else:
                nc.vector.tensor_tensor(out[:, dt, :], y[:, dt, :].bitcast(F32),
                                        bc1[:], op=OP.mult)
                nc.vector.tensor_scalar(out[:, dt, :], out[:, dt, :],
                                        g_col[:, dt:dt + 1], b_col[:, dt:dt + 1],
                                        OP.mult, OP.add)

    # =======================================================================
    # embedding: gather + add + transpose to feature-major + LN -> xT
    with tc.tile_pool(name="emb_sb", bufs=4) as embp:
        egc = ppool.tile([128, NT], F32, name="egc")
        nc.sync.dma_start(egc[:], d["d_eg"].ap().rearrange("(n p) -> p n", p=128))
        ebc = ppool.tile([128, NT], F32, name="ebc")
        nc.sync.dma_start(ebc[:], d["d_eb"].ap().rearrange("(n p) -> p n", p=128))
        for st in range(NST):
            idst = embp.tile([128, 1], I32, name="idst", tag="idst")
            nc.sync.dma_start(idst[:], d["d_ids"].ap()[st * 128:(st + 1) * 128, :])
            ttst = embp.tile([128, 1], I32, name="ttst", tag="ttst")
            nc.sync.dma_start(ttst[:], d["d_tti"].ap()[st * 128:(st + 1) * 128, :])
            x0 = embp.tile([128, D], F32, name="x0", tag="x0")
            nc.gpsimd.indirect_dma_start(
                out=x0[:], out_offset=None, in_=d["d_wemb"].ap(),
                in_offset=bass.IndirectOffsetOnAxis(ap=idst[:, :1], axis=0))
            tg = embp.tile([128, D], F32, name="tg", tag="tg")
            nc.gpsimd.indirect_dma_start(
                out=tg[:], out_offset=None, in_=d["d_temb"].ap(),
                in_offset=bass.IndirectOffsetOnAxis(ap=ttst[:, :1], axis=0))
            pg = embp.tile([128, D], F32, name="pg", tag="pg")
            nc.sync.dma_start(pg[:], d["d_pemb"].ap()[st * 128:(st + 1) * 128, :])
            nc.vector.tensor_tensor(x0[:], x0[:], tg[:], op=OP.add)
            nc.vector.tensor_tensor(x0[:], x0[:], pg[:], op=OP.add)
            for dt in range(NT):
                trp = ps_s.tile([128, 128], F32, name="trp", tag="s_rot")
                nc.tensor.transpose(trp[:], x0[:, dt * 128:(dt + 1) * 128], ident[:])
                nc.vector.tensor_copy(ybuf[:, dt, st * 128:(st + 1) * 128], trp[:])
        est0 = ps_a.tile([1, S], F32, name="est0", tag="a0")
        est1 = ps_a.tile([1, S], F32, name="est1", tag="a1")
        ebc0 = ps_s.tile([128, S], F32, name="ebc0", tag="s_rot")
        ebc1 = ps_s.tile([128, S], F32, name="ebc1", tag="s_rot")
        layernorm(ybuf, NT, 1e-12, xT, D, est0, est1, ebc0, ebc1,
                  g_col=egc[:], b_col=ebc[:])

    # =======================================================================
    # transformer layers
    for l in range(L):
        # ---- per-layer params -------------------------------------------
        bqc = ppool.tile([128, NT], F32, name="bqc", tag="bqc")
        nc.sync.dma_start(bqc[:], d["d_bq"].ap()[l].rearrange("(n p) -> p n", p=128))
        bkc = ppool.tile([128, NT], F32, name="bkc", tag="bkc")
        nc.sync.dma_start(bkc[:], d["d_bk"].ap()[l].rearrange("(n p) -> p n", p=128))
        bvr = ppool.tile([1, D], F32R, name="bvr", tag="bvr")
        nc.sync.dma_start(bvr[:], d["d_bv"].ap()[l].rearrange("(o e) -> o e", o=1).bitcast(F32R))
        boc = ppool.tile([128, NT], F32, name="boc", tag="boc")
        nc.sync.dma_start(boc[:], d["d_bo"].ap()[l].rearrange("(n p) -> p n", p=128))
        b2c = ppool.tile([128, NT], F32, name="b2c", tag="b2c")
        nc.sync.dma_start(b2c[:], d["d_b2"].ap()[l].rearrange("(n p) -> p n", p=128))
        fgc = ppool.tile([128, NT], F32, name="fgc", tag="fgc")
        nc.sync.dma_start(fgc[:], d["d_fg"].ap()[l].rearrange("(n p) -> p n", p=128))
        fbc = ppool.tile([128, NT], F32, name="fbc", tag="fbc")
        nc.sync.dma_start(fbc[:], d["d_fb"].ap()[l].rearrange("(n p) -> p n", p=128))

        # ---- phase 1: interleaved QKV + attention + Wo -------------------
        e_tiles = [None] * NP

        def emit_v(c):
            wv = wvpool.tile([128, NT, 256], BF16, name=f"wv{c}", tag="wv")
            nc.sync.dma_start(wv[:], d["d_wv"].ap()[l, c])
            for st in range(NST):
                acc = ps_a.tile([128, 256], F32, name=f"v{st}", tag=f"a{st % 2}")
                for dt in range(NT):
                    nc.tensor.matmul(acc[:], xT[:, dt, st * 128:(st + 1) * 128],
                                     wv[:, dt, :],
                                     start=(dt == 0), stop=False)
                nc.tensor.matmul(acc[:], one_row[:],
                                 bvr[:, c * 256:(c + 1) * 256],
                                 start=False, stop=True, skip_group_check=True)
                nc.vector.tensor_copy(
                    vaug[:, st, c * 4:(c + 1) * 4, 0:DK],
                    acc[:].rearrange("p (a b) -> p a b", a=4))

        def emit_qk(et):
            for (wd, bcol, dst, nm) in ((d["d_wq"], bqc, qT, "q"),
                                        (d["d_wk"], bkc, kT, "k")):
                wt = wqkpool.tile([128, NT, 128], BF16, name=f"w{nm}{et}", tag="wqk")
                nc.sync.dma_start(wt[:], wd.ap()[l, et])
                acc = ps_a.tile([128, S], F32, name=f"{nm}{et}", tag=f"a{et % 2}")
                for dt in range(NT):
                    nc.tensor.matmul(acc[:], wt[:, dt, :], xT[:, dt, :],
                                     start=(dt == 0), stop=(dt == NT - 1))
                nc.vector.tensor_scalar(dst[:, et, :], acc[:],
                                        bcol[:, et:et + 1], None, OP.add)

        def emit_scores(p):
            ets = []
            for kt in range(NST):
                scs = []
                for hh in range(2):
                    lo = hh * 64
                    sc = ps_s.tile([128, S], F32, name=f"sc{kt}{hh}", tag="s_rot")
                    nc.tensor.matmul(
                        sc[:], kT[lo:lo + 64, p, kt * 128:(kt + 1) * 128],
                        qT[lo:lo + 64, p, :], start=True, stop=True,
                        tile_position=(lo, 0))
                    scs.append(sc)
                for hh in range(2):
                    et = epool.tile([128, S], BF16, name=f"e{kt}{hh}", tag="e")
                    nc.scalar.activation(et[:], scs[hh][:], AF.Exp,
                                         bias=maskc[:, kt:kt + 1])
                    ets.append(et)
            e_tiles[p] = ets

        def emit_pv(p):
            ets = e_tiles[p]
            cpss = []
            for hh in range(2):
                h = 2 * p + hh
                cps = ps_c.tile([DK + 1, S], F32, name=f"cps{hh}", tag=f"c{hh}")
                for kt in range(NST):
                    nc.tensor.matmul(cps[:],
                                     vaug[:, kt, h, 0:DK + 1],
                                     ets[kt * 2 + hh][:],
                                     start=(kt == 0), stop=(kt == NST - 1))
                cpss.append(cps)
            nc.vector.tensor_copy(dpair[0:1, :], cpss[0][DK:DK + 1, :])
            nc.vector.tensor_copy(dpair[32:33, :], cpss[1][DK:DK + 1, :])
            nl33 = spool.tile([33, S], F32, name="nl33", tag="nl33")
            nc.scalar.activation(nl33[:], dpair[:], AF.Ln)
            nc.scalar.activation(r01[:], nl33[:], AF.Exp, scale=-1.0)
            rps = ps_r.tile([128, S], F32, name="rps", tag="r0")
            nc.tensor.matmul(rps[:], sel2[:], r01[:], start=True, stop=True)
            rsb = spool.tile([128, S], F32, name="rsb", tag=f"rsb{p % 2}")
            nc.vector.tensor_copy(rsb[:], rps[:])
            for hh in range(2):
                lo = hh * 64
                nc.vector.tensor_tensor(cT[lo:lo + DK, p, :], cpss[hh][0:DK, :],
                                        rsb[lo:lo + DK, :], op=OP.mult)

        def emit_wo(et, st0, st1):
            wt = wopool.tile([128, NT, 128], BF16, name=f"wo{et}", tag="wo")
            nc.sync.dma_start(wt[:], d["d_wo"].ap()[l, et])
            acc = ps_a.tile([128, S], F32, name=f"o{et}", tag=f"a{et % 2}")
            for dt in range(NT):
                nc.tensor.matmul(acc[:], wt[:, dt, :], cT[:, dt, :],
                                 start=(dt == 0), stop=(dt == NT - 1))
            nc.vector.scalar_tensor_tensor(
                ybuf[:, et, :], acc[:], boc[:, et:et + 1],
                xT[:, et, :], op0=OP.add, op1=OP.add)
            # LN1 stats interleaved: square + partial sums as tiles land
            sqt = spool.tile([128, S], BF16, name=f"sq{et}", tag=f"sq{et % 2}")
            nc.scalar.activation(sqt[:], ybuf[:, et, :].bitcast(F32), AF.Square)
            nc.tensor.matmul(st0[:], ones_col[:], ybuf[:, et, :],
                             start=(et == 0), stop=(et == NT - 1),
                             skip_group_check=True)
            nc.tensor.matmul(st1[:], ones_colb[:], sqt[:],
                             start=(et == 0), stop=(et == NT - 1),
                             skip_group_check=True)

        # interleaved emission
        emit_v(0)
        emit_qk(0)
        emit_qk(1)
        emit_scores(0)
        emit_v(1)
        emit_qk(2)
        emit_scores(1)
        emit_pv(0)
        emit_v(2)
        emit_qk(3)
        emit_scores(2)
        emit_pv(1)
        emit_qk(4)
        emit_scores(3)
        emit_pv(2)
        emit_qk(5)
        emit_scores(4)
        emit_pv(3)
        emit_scores(5)
        emit_pv(4)
        emit_pv(5)

        # ---- Wo + residual + LN1 (gamma/beta folded into W1) -------------
        lst0 = ps_s.tile([1, S], F32, name="lst0", tag="s_rot")
        lst1 = ps_s.tile([1, S], F32, name="lst1", tag="s_rot")
        for et in range(NT):
            emit_wo(et, lst0, lst1)
        lbc0 = ps_s.tile([128, S], F32, name="lbc0", tag="s_rot")
        lbc1 = ps_c.tile([128, S], F32, name="lbc1", tag="c0")
        # LN1 chain (sums already accumulated inside emit_wo)
        mu = spool.tile([1, S], F32R, name="mu", tag="ln_mu")
        nc.vector.tensor_scalar(mu[:], lst0[:], 1.0 / D, None, OP.mult)
        nc.tensor.matmul(lbc0[:], neg_row[:], mu[:], start=True, stop=True)
        msq = spool.tile([1, S], F32R, name="msq", tag="ln_msq")
        nc.vector.tensor_scalar(msq[:], lst1[:], 1.0 / D, 1e-5, OP.mult, OP.add)
        nc.tensor.matmul(lbc1[:], one_row[:], msq[:], start=True, stop=True)
        musq = spool.tile([1, S], F32R, name="musq", tag="ln_musq")
        nc.vector.tensor_tensor(musq[:], mu[:].bitcast(F32), mu[:].bitcast(F32), op=OP.mult)
        nc.tensor.matmul(lbc1[:], one_row[:], musq[:], start=True, stop=True)
        var = spool.tile([1, S], F32R, name="var", tag="ln_var")
        nc.vector.tensor_tensor(var[:], msq[:].bitcast(F32), musq[:].bitcast(F32), op=OP.subtract)
        nc.tensor.matmul(lbc1[:], one_row[:], var[:], start=True, stop=True)
        lnv = spool.tile([1, S], F32R, name="lnv", tag="ln_lnv")
        nc.scalar.activation(lnv[:], var[:].bitcast(F32), AF.Ln)
        nc.tensor.matmul(lbc1[:], one_row[:], lnv[:], start=True, stop=True)
        rstd = spool.tile([1, S], F32R, name="rstd", tag="ln_rstd")
        nc.scalar.activation(rstd[:], lnv[:].bitcast(F32), AF.Exp, scale=-0.5)
        nc.tensor.matmul(lbc1[:], one_row[:], rstd[:], start=True, stop=True)
        for dt in range(NT):
            nc.vector.tensor_tensor(ybuf[:, dt, :], ybuf[:, dt, :].bitcast(F32),
                                    lbc0[:], op=OP.add)
            nc.vector.tensor_tensor(aT[:, dt, :], ybuf[:, dt, :].bitcast(F32),
                                    lbc1[:], op=OP.mult)

        # ---- phase 4: FFN (W1 -> h, W2 accumulate into 6 yT banks) -------
        YT = ["s_rot", "s_rot", "s_rot", "c0", "c1", "r0"]
        ytiles = []
        for et in range(NT):
            yt = (ps_s if YT[et].startswith("s") else (ps_c if YT[et].startswith("c") else ps_r)
                  ).tile([128, S], F32, name=f"yt{et}", tag=YT[et])
            ytiles.append(yt)
        h_sb = [None] * NTF
        w2ts = [None] * NTF

        def emit_h(f):
            c, fj = divmod(f, 2)
            if fj == 0:
                w1t = w1pool.tile([128, NT, 256], BF16, name=f"w1_{c}", tag="w1")
                nc.sync.dma_start(w1t[:], d["d_w1"].ap()[l, c])
                emit_h.w1t = w1t
                w2t = w2pool.tile([128, 2, D], BF16, name=f"w2_{c}", tag="w2")
                nc.sync.dma_start(w2t[:], d["d_w2"].ap()[l, c])
                emit_h.w2t = w2t
            hacc = ps_a.tile([128, S], F32, name=f"h{f}", tag=f"a{f % 2}")
            for dt in range(NT):
                nc.tensor.matmul(hacc[:], emit_h.w1t[:, dt, fj * 128:(fj + 1) * 128],
                                 aT[:, dt, :], start=(dt == 0), stop=(dt == NT - 1))
            hs = hpool.tile([128, S], BF16, name=f"hs{f}", tag="hs")
            nc.vector.tensor_copy(hs[:], hacc[:])
            h_sb[f] = hs
            w2ts[f] = emit_h.w2t

        def emit_y(f):
            fj = f % 2
            w2t = w2ts[f]
            for et in range(NT):
                nc.tensor.matmul(ytiles[et][:],
                                 w2t[:, fj, et * 128:(et + 1) * 128],
                                 h_sb[f][:], start=(f == 0), stop=(f == NTF - 1),
                                 skip_group_check=True)
            h_sb[f] = None

        emit_h(0)
        for f in range(1, NTF):
            emit_h(f)
            emit_y(f - 1)
        emit_y(NTF - 1)

        # epilogue: +b2'' (per-partition) -> ybuf
        for et in range(NT):
            nc.vector.tensor_scalar(ybuf[:, et, :], ytiles[et][:],
                                    b2c[:, et:et + 1], None, OP.add)

        # ---- LN2 -> xT (next layer input) --------------------------------
        nst0 = ps_a.tile([1, S], F32, name="nst0", tag="a0")
        nst1 = ps_a.tile([1, S], F32, name="nst1", tag="a1")
        nbc0 = ps_s.tile([128, S], F32, name="nbc0", tag="s_rot")
        nbc1 = ps_s.tile([128, S], F32, name="nbc1", tag="s_rot")
        layernorm(ybuf, NT, 1e-5, xT, D, nst0, nst1, nbc0, nbc1,
                  g_col=fgc[:], b_col=fbc[:])

    # =======================================================================
    # output: transpose xT -> [S, D] and DMA out
    with tc.tile_pool(name="out_sb", bufs=2) as outp:
        for st in range(NST):
            ops_t = ps_s.tile([128, D], BF16, name="ops", tag="s_rot")
            for dt in range(NT):
                nc.tensor.transpose(ops_t[:, dt * 128:(dt + 1) * 128],
                                    xT[:, dt, st * 128:(st + 1) * 128],
                                    ident16[:])
            osb = outp.tile([128, D], F32, name="osb", tag="osb")
            nc.vector.tensor_copy(osb[:], ops_t[:])
            nc.sync.dma_start(d["d_out"].ap()[st * 128:(st + 1) * 128, :], osb[:])


# ---------------------------------------------------------------------------
def _sel2():
    s = np.zeros((33, 128), np.float32)
    s[0, 0:64] = 1.0
    s[32, 64:128] = 1.0
    return s


def kernel(**inputs):
    global _NC_CACHE
    if _NC_CACHE is None:
        _NC_CACHE = _build_nc()
    nc = _NC_CACHE

    import ml_dtypes
    f32 = lambda a: np.ascontiguousarray(np.asarray(a), dtype=np.float32)
    bf = lambda a: np.ascontiguousarray(a.astype(ml_dtypes.bfloat16))

    Wq = f32(inputs["Wq"]) * SCALE
    bq = f32(inputs["bq"]) * SCALE
    Wk, Wv, Wo = f32(inputs["Wk"]), f32(inputs["Wv"]), f32(inputs["Wo"])
    W1, W2 = f32(inputs["W1"]), f32(inputs["W2"])
    ag, ab = f32(inputs["attn_ln_g"]), f32(inputs["attn_ln_b"])
    b1, b2 = f32(inputs["b1"]), f32(inputs["b2"])
    # fold LN1 gamma into W1 columns, beta into b1; then b1 into b2
    W1f = W1 * ag[:, None, :]                    # [L,F,D] * [L,1,D]
    b1f = b1 + np.einsum("lfd,ld->lf", W1, ab)
    b2f = b2 + np.einsum("ldf,lf->ld", W2, b1f)

    def tile_qk(W):  # [L, Dout, Din] -> [L, et, p, n, e]
        WT = W.transpose(0, 2, 1)                # [L, Din, Dout]
        return bf(WT.reshape(L, NT, 128, NT, 128).transpose(0, 3, 2, 1, 4))

    def tile_v(W):   # -> [L, c3, p, n, e256]
        WT = W.transpose(0, 2, 1)
        return bf(WT.reshape(L, NT, 128, 3, 256).transpose(0, 3, 2, 1, 4))

    def tile_w1(W):  # [L, F, D] -> [L, c12, p, n6, e256]
        WT = W.transpose(0, 2, 1)                # [L, D, F]
        return bf(WT.reshape(L, NT, 128, NTF // 2, 256).transpose(0, 3, 2, 1, 4))

    def tile_w2(W):  # [L, D, F] -> [L, c12, p, g2, e768]
        WT = W.transpose(0, 2, 1)                # [L, F, D]
        return bf(WT.reshape(L, NTF // 2, 2, 128, D).transpose(0, 1, 3, 2, 4))

    shared = {
        "wemb": f32(inputs["word_emb"]),
        "pemb": f32(inputs["pos_emb"])[:S],
        "temb": f32(inputs["type_emb"]),
        "eg": f32(inputs["emb_ln_g"]), "eb": f32(inputs["emb_ln_b"]),
        "wq": tile_qk(Wq),
        "wk": tile_qk(Wk),
        "wv": tile_v(Wv),
        "wo": tile_qk(Wo),
        "w1": tile_w1(W1f),
        "w2": tile_w2(W2),
        "bq": bq, "bk": f32(inputs["bk"]), "bv": f32(inputs["bv"]),
        "bo": f32(inputs["bo"]), "b2": b2f,
        "fg": f32(inputs["ffn_ln_g"]), "fb": f32(inputs["ffn_ln_b"]),
        "ones128": np.ones(128, np.float32),
        "ones128b": np.ones(128, ml_dtypes.bfloat16),
        "ones512": np.ones((1, 512), np.float32),
        "neg1": np.full((1, 128), -1.0, np.float32),
        "sel2": _sel2(),
        "onesgridb": np.ones((128, NST * H), ml_dtypes.bfloat16),
    }
    ids = np.asarray(inputs["input_ids"]).astype(np.int32)
    tti = np.asarray(inputs["token_type_ids"]).astype(np.int32)
    am = np.asarray(inputs["attention_mask"]).astype(np.float32)
    in_maps = []
    for c in range(B):
        in_maps.append({
            **shared,
            "ids": ids[c].reshape(S, 1),
            "tti": tti[c].reshape(S, 1),
            "maskadd": np.where(am[c] == 0, -1e9, 0.0).astype(np.float32),
        })
    res = bass_utils.run_bass_kernel_spmd(
        nc, in_maps, core_ids=list(range(B)), trace=False)
    out = np.stack([res.results[c]["out"] for c in range(B)], axis=0)
    return out.astype(np.float32)
